# revision 25
# baseline (speedup 1.0000x reference)
"""Trainium2 Bass kernel for a 2-layer LSTM decoder + vocab projection + log-softmax.

Parallelization (8 cores, symmetric SPMD):
  - Each core owns a 1024-row slice of the 8192 LSTM gate rows of BOTH layers
    (256 rows from each of the i,f,o,g blocks -> core c owns h-dims
    [256c, 256c+256)), and a 4000-col slice of the vocab projection.
  - All weights live in SBUF as fp8e4m3 (scaled by powers of 2 into fp8's
    normal range; compensated via activation scales). Activations stay bf16.
  - Per tick: h @ W_hh.T slice -> LSTM cell -> merged AllGather of the two
    layers' 256-dim h slices. Layer 1 lags LAG ticks; its input matmul
    batches D steps per weight pass.
  - The full vocab projection (fp8 W_out fully SBUF-resident), the softmax
    normalizer AllReduce (sum only; logits are small enough to skip the max
    shift) and the final logp = logits - logZ output writes are interleaved
    into the recurrence ticks.
  - LSTM gates use tanh only (sigmoid(x) = 0.5 tanh(x/2) + 0.5, folded into
    host-side weight scaling) so the scalar engine never swaps activation
    tables in steady state. h is stored at 2x scale (absorbed into weights).
"""

import numpy as np
import ml_dtypes
from contextlib import ExitStack

import concourse.bass as bass
import concourse.mybir as mybir
import concourse.tile as tile
from concourse import bacc
from concourse import bass_utils

F32 = mybir.dt.float32
F16 = mybir.dt.float16
BF16 = mybir.dt.bfloat16
FP8 = mybir.dt.float8e4
AF = mybir.ActivationFunctionType
ALU = mybir.AluOpType
bf16 = ml_dtypes.bfloat16
f8 = ml_dtypes.float8_e4m3fn

H = 1024
RH = 2048
V = 32000
B = 32
T = 128
NC = 8
GS = 1024          # gate rows per core per layer
HS = 256           # h dims per core
VS = V // NC       # vocab cols per core
D = 4              # layer-1 input-matmul batching (steps per weight pass)
LAG = D + 1
SOS_ID = 1
SC = 32.0          # fp8 weight upscale (power of 2: exact)
SX = 32.0          # embedding upscale
SC0 = 32.0         # Wih0 upscale

TS_FULL = T - 1    # 127 recurrence steps
DBG = False        # add j=0 debug dumps (debug_ts.py flips this)


def _wpack(w, kt):
    """w: [K=kt*128, M] -> [128, kt*M] fp8, k-tile-major."""
    K, M = w.shape
    assert K == kt * 128
    return np.ascontiguousarray(
        w.reshape(kt, 128, M).transpose(1, 0, 2).reshape(128, kt * M)
    ).astype(f8)


def prep_inputs(inp, ts=TS_FULL):
    ntok = ts * B
    ntok_pad = ((ntok + 127) // 128) * 128
    f32 = np.float32

    emb = np.asarray(inp["emb"], f32)
    tb = np.asarray(inp["target_batch"]).astype(np.int64)
    x = emb[tb[:, :ts]]                       # (B, ts, H)
    x = x.transpose(1, 0, 2).reshape(ntok, H)
    xp = np.zeros((ntok_pad, H), f32)
    xp[:ntok] = x * SX
    xt = np.ascontiguousarray(
        xp.T.reshape(8, 128, ntok_pad).transpose(1, 0, 2)
        .reshape(128, 8 * ntok_pad)).astype(f8)

    ch = np.asarray(inp["context_h"], f32)
    cc = np.asarray(inp["context_c"], f32)
    h_init = np.concatenate([ch[0::2], ch[1::2]], axis=2)  # (2, B, RH)
    c_init = np.concatenate([cc[0::2], cc[1::2]], axis=2)

    def h_pack(hl):  # (B, RH) -> [128, 16*32] bf16, k-major, 2x scale
        return np.ascontiguousarray(
            (2.0 * hl).T.reshape(16, 128, B).transpose(1, 0, 2).reshape(128, 16 * B)
        ).astype(bf16)

    Wih = [np.asarray(inp["W_ih0"], f32), np.asarray(inp["W_ih1"], f32)]
    Whh = [np.asarray(inp["W_hh0"], f32), np.asarray(inp["W_hh1"], f32)]
    bsum = [np.asarray(inp["b_ih0"], f32) + np.asarray(inp["b_hh0"], f32),
            np.asarray(inp["b_ih1"], f32) + np.asarray(inp["b_hh1"], f32)]
    W_out = np.asarray(inp["W_out"], f32)
    b_out = np.asarray(inp["b_out"], f32)

    in_maps = []
    for c in range(NC):
        rows = np.concatenate([np.arange(RH * k + HS * c, RH * k + HS * (c + 1))
                               for k in (0, 1, 3, 2)])  # i,f,o,g
        rs = np.concatenate([np.full(768, 0.5, f32), np.ones(256, f32)])
        wih0 = _wpack((SC0 * rs[None, :] * Wih[0][rows].T), 8)
        whh0 = _wpack((SC * 0.5 * rs[None, :] * Whh[0][rows].T), 16)
        wih1 = _wpack((SC * 0.5 * rs[None, :] * Wih[1][rows].T), 16)
        whh1 = _wpack((SC * 0.5 * rs[None, :] * Whh[1][rows].T), 16)
        wout = _wpack((SC * 0.5 * W_out[VS * c:VS * (c + 1)].T), 16)
        b0 = np.ascontiguousarray((SC * rs * bsum[0][rows]).reshape(8, 128).T)
        b1 = np.ascontiguousarray((SC * rs * bsum[1][rows]).reshape(8, 128).T)
        boutc = (SC * b_out[VS * c:VS * (c + 1)]).reshape(1, VS).astype(bf16)

        def c_pack(cl):  # (B, RH) slice -> [128, 64] f32, natural scale
            s = cl[:, HS * c:HS * (c + 1)].T
            return np.ascontiguousarray(
                s.reshape(2, 128, B).transpose(1, 0, 2).reshape(128, 2 * B))

        in_maps.append({
            "xt": xt,
            "wih0": wih0, "whh0": whh0, "wih1": wih1, "whh1": whh1,
            "wout": wout, "b0": b0, "b1": b1, "boutc": boutc,
            "id128": np.eye(128).astype(bf16),
            "h0i": h_pack(h_init[0]), "h1i": h_pack(h_init[1]),
            "c0i": c_pack(c_init[0]), "c1i": c_pack(c_init[1]),
        })
    return in_maps, ntok_pad


def build_nc(ts=TS_FULL):
    ntok = ts * B
    ntok_pad = ((ntok + 127) // 128) * 128
    ngrp = ntok_pad // 128
    nticks = ts + LAG + 1
    NCH = (ntok_pad + 511) // 512        # 512-token phase-0 chunks

    nc = bacc.Bacc("TRN2", target_bir_lowering=False, debug=False,
                   enable_asserts=False, num_devices=NC)

    xt_t = nc.dram_tensor("xt", [128, 8 * ntok_pad], FP8, kind="ExternalInput").ap()
    wih0_t = nc.dram_tensor("wih0", [128, 8 * GS], FP8, kind="ExternalInput").ap()
    whh0_t = nc.dram_tensor("whh0", [128, 16 * GS], FP8, kind="ExternalInput").ap()
    wih1_t = nc.dram_tensor("wih1", [128, 16 * GS], FP8, kind="ExternalInput").ap()
    whh1_t = nc.dram_tensor("whh1", [128, 16 * GS], FP8, kind="ExternalInput").ap()
    wout_t = nc.dram_tensor("wout", [128, 16 * VS], FP8, kind="ExternalInput").ap()
    b0_t = nc.dram_tensor("b0", [128, 8], F32, kind="ExternalInput").ap()
    b1_t = nc.dram_tensor("b1", [128, 8], F32, kind="ExternalInput").ap()
    bout_t = nc.dram_tensor("boutc", [1, VS], BF16, kind="ExternalInput").ap()
    id_t = nc.dram_tensor("id128", [128, 128], BF16, kind="ExternalInput").ap()
    h0i_t = nc.dram_tensor("h0i", [128, 16 * B], BF16, kind="ExternalInput").ap()
    h1i_t = nc.dram_tensor("h1i", [128, 16 * B], BF16, kind="ExternalInput").ap()
    c0i_t = nc.dram_tensor("c0i", [128, 64], F32, kind="ExternalInput").ap()
    c1i_t = nc.dram_tensor("c1i", [128, 64], F32, kind="ExternalInput").ap()
    out_t = nc.dram_tensor("out", [ntok_pad, VS], F16, kind="ExternalOutput").ap()
    if DBG:
        dps1_t = nc.dram_tensor("dps1", [128, 256], F32, kind="ExternalOutput").ap()
        dg1_t = nc.dram_tensor("dg1", [128, 256], F32, kind="ExternalOutput").ap()
        dh1_t = nc.dram_tensor("dh1", [128, 64], F32, kind="ExternalOutput").ap()
        dps0_t = nc.dram_tensor("dps0", [128, 256], F32, kind="ExternalOutput").ap()
        dh0_t = nc.dram_tensor("dh0", [128, 16 * B], F32, kind="ExternalOutput").ap()

    RG = [list(range(NC))]

    full_groups = [g for g in range(ngrp) if 4 * g + 3 <= ts - 1]
    proj_inloop = {}
    proj_tail = []
    for g in range(ngrp):
        for q in range(4):
            tk = 4 * g + 10 + q
            if g in full_groups and tk <= nticks - 1:
                proj_inloop.setdefault(tk, []).append((g, q))
            else:
                proj_tail.append((g, q))
    nbatch = (ngrp + 7) // 8
    ar_inloop = {}
    ar_tail = []
    passb_inloop = {}
    for b in range(nbatch):
        gs = list(range(8 * b, min(8 * b + 8, ngrp)))
        ar_tick = max(4 * g + 13 for g in gs) + 1
        ok = all(g in full_groups
                 and all((4 * g + 10 + q) in proj_inloop for q in range(4))
                 for g in gs)
        if ok and ar_tick + 3 + len(gs) <= nticks - 1:
            ar_inloop[ar_tick] = b
            for i, g in enumerate(gs):
                passb_inloop.setdefault(ar_tick + 3 + i, []).append(g)
        else:
            ar_tail.append(b)

    with ExitStack() as ctx:
        tc = ctx.enter_context(tile.TileContext(nc))
        dram = ctx.enter_context(tc.tile_pool(name="dram", bufs=1, space="DRAM"))
        agp = ctx.enter_context(tc.tile_pool(name="agp", bufs=6, space="DRAM"))
        keep = ctx.enter_context(tc.tile_pool(name="keep", bufs=1))

        g0_d = dram.tile([128, (ntok_pad // B) * 8 * B], BF16, tag="g0d")
        logits_d = dram.tile([ngrp, 128, VS], F16, tag="logitsd")

        s4 = keep.tile([128, ngrp * 4], F32, tag="s4")
        s_all = keep.tile([128, ngrp], F32, tag="sall")
        logz = keep.tile([128, ngrp], F32, tag="logz")
        ones_s = keep.tile([1, 128], BF16, tag="ones")
        bout_s = keep.tile([1, VS], BF16, tag="bouts")
        id_s = keep.tile([128, 128], BF16, tag="ids")

        with tc.tile_pool(name="rp", bufs=1) as rp, \
             tc.tile_pool(name="rings", bufs=1) as rngp, \
             tc.tile_pool(name="stgp", bufs=1) as stgp, \
             tc.tile_pool(name="osbp", bufs=2) as osbp, \
             tc.tile_pool(name="xts", bufs=2) as xtsp, \
             tc.tile_pool(name="g0ev", bufs=2) as g0ev, \
             tc.tile_pool(name="cellp", bufs=2) as cellp, \
             tc.tile_pool(name="projp", bufs=2) as projp, \
             tc.tile_pool(name="pbp", bufs=1 if DBG else 2) as pbp, \
             tc.tile_pool(name="arp", bufs=2) as arp, \
             tc.tile_pool(name="psA", bufs=1, space="PSUM") as psA, \
             tc.tile_pool(name="psB", bufs=1, space="PSUM") as psB, \
             tc.tile_pool(name="psG", bufs=1, space="PSUM") as psG, \
             tc.tile_pool(name="psP", bufs=2, space="PSUM") as psP:

            wih0_s = rp.tile([128, 8 * GS], FP8, tag="wih0")
            whh0_s = rp.tile([128, 16 * GS], FP8, tag="whh0")
            wih1_s = rp.tile([128, 16 * GS], FP8, tag="wih1")
            whh1_s = rp.tile([128, 16 * GS], FP8, tag="whh1")
            wout_s = rp.tile([128, 16 * VS], FP8, tag="wout")
            b0_s = rp.tile([128, 8], F32, tag="b0")
            b1_s = rp.tile([128, 8], F32, tag="b1")
            h0i_s = rp.tile([128, 16 * B], BF16, tag="h0i")
            h1i_s = rp.tile([128, 16 * B], BF16, tag="h1i")

            h0ring = rngp.tile([128, 16 * 8 * B], BF16, tag="h0r")   # [k,s,b]
            g0ring = rngp.tile([128, 2 * 8 * 256], BF16, tag="g0r")  # [half,s,(m b)]
            g1ring = rngp.tile([128, 4 * 256], BF16, tag="g1r")      # [s,(m b)]
            h0r = h0ring[:].rearrange("p (k s b) -> p k s b", k=16, b=B)
            h0r5 = h0ring[:].rearrange("p (kp j s b) -> p kp j s b",
                                       kp=8, j=2, b=B)
            g0r = g0ring[:].rearrange("p (h s mb) -> p h s mb", h=2, mb=256)
            g1r = g1ring[:].rearrange("p (s mb) -> p s mb", mb=256)
            h0i4 = h0i_s[:].rearrange("p (k b) -> p k b", b=B)
            h1i4 = h1i_s[:].rearrange("p (k b) -> p k b", b=B)

            stg0 = stgp.tile([128, 128], BF16, tag="stg0")
            stg1 = stgp.tile([128, 128], BF16, tag="stg1")
            stg = [stg0, stg1]

            nc.sync.dma_start(id_s[:], id_t[:])
            nc.sync.dma_start(b0_s[:], b0_t[:])
            nc.sync.dma_start(wih0_s[:], wih0_t[:])
            nc.sync.dma_start(whh0_s[:], whh0_t[:])
            nc.sync.dma_start(h0i_s[:], h0i_t[:])
            nc.scalar.dma_start(b1_s[:], b1_t[:])
            nc.scalar.dma_start(wih1_s[:], wih1_t[:])
            nc.scalar.dma_start(whh1_s[:], whh1_t[:])
            nc.scalar.dma_start(h1i_s[:], h1i_t[:])
            nc.scalar.dma_start(wout_s[:], wout_t[:])
            nc.scalar.dma_start(bout_s[:], bout_t[:])
            nc.gpsimd.memset(ones_s[:], 1.0)
            nc.gpsimd.memset(stg[0][:], 0.0)
            nc.gpsimd.memset(stg[1][:], 0.0)

            wih0v = wih0_s[:].rearrange("p (k m) -> p k m", k=8)
            whh0v = whh0_s[:].rearrange("p (k m) -> p k m", k=16)
            wih1v = wih1_s[:].rearrange("p (k m) -> p k m", k=16)
            whh1v = whh1_s[:].rearrange("p (k m) -> p k m", k=16)
            woutv = wout_s[:].rearrange("p (k m) -> p k m", k=16)

            g0_dv = g0_d[:].rearrange("p (s m b) -> p s m b", m=8, b=B)

            def g0_chunk(ci):
                t0 = 512 * ci
                valid = min(512, ntok - t0)
                if valid <= 0:
                    return
                s0 = 16 * ci
                vsteps = valid // B
                cw = min(512, ntok_pad - t0)
                xts = xtsp.tile([128, 8 * 512], FP8, tag="xts")
                xtv = xts[:].rearrange("p (k c) -> p k c", k=8)
                nc.sync.dma_start(
                    xtv[:, :, 0:cw], xt_t[:].rearrange("p (k c) -> p k c", k=8)
                    [:, :, t0:t0 + cw])
                for m in range(8):
                    ps = psP.tile([128, 1024], F32, tag="psP")
                    for k in range(8):
                        nc.tensor.matmul(
                            ps[:, 0:cw], wih0v[:, k, 128 * m:128 * (m + 1)],
                            xtv[:, k, 0:cw], start=(k == 0), stop=(k == 7))
                    ev = g0ev.tile([128, 512], BF16, tag="g0ev")
                    nc.scalar.activation(ev[:, :valid], ps[:, :valid],
                                         AF.Identity, bias=b0_s[:, m:m + 1],
                                         scale=float(SC / (SC0 * SX)))
                    nc.scalar.dma_start(
                        g0_dv[:, s0:s0 + vsteps, m, :],
                        ev[:, :B * vsteps].rearrange("p (s b) -> p s b", b=B))

            def g0_prefetch(blk):
                t0 = 8 * blk
                nsteps = min(8, ts - t0)
                if nsteps <= 0:
                    return
                nc.scalar.dma_start(
                    g0r[:, blk % 2, 0:nsteps, :],
                    g0_dv[:, t0:t0 + nsteps, :, :].rearrange(
                        "p s m b -> p s (m b)"))

            def hh(wv, rhs_of_k, ps, gadd):
                for m in range(8):
                    for k in range(16):
                        nc.tensor.matmul(
                            ps[:, 32 * m:32 * (m + 1)],
                            wv[:, k, 128 * m:128 * (m + 1)],
                            rhs_of_k(k), start=(k == 0), stop=False)
                    nc.tensor.matmul(ps[:, 32 * m:32 * (m + 1)], id_s[:],
                                     gadd[:, 32 * m:32 * (m + 1)],
                                     start=False, stop=True)

            c_prev = [None, None]
            c0s = cellp.tile([128, 64], F32, tag="c0i")
            nc.sync.dma_start(c0s[:], c0i_t[:])
            c_prev[0] = c0s
            c1s = cellp.tile([128, 64], F32, tag="c1i")
            nc.sync.dma_start(c1s[:], c1i_t[:])
            c_prev[1] = c1s

            def cell(l, ps, stg_out):
                tact = cellp.tile([128, 256], F32, tag=f"ta{l}")
                nc.scalar.activation(tact[:], ps[:, 0:256], AF.Tanh,
                                     scale=float(1.0 / SC))
                t1 = cellp.tile([128, 64], F32, tag=f"t1{l}")
                nc.vector.scalar_tensor_tensor(
                    t1[:], tact[:, 0:64], 1.0, tact[:, 192:256],
                    op0=ALU.add, op1=ALU.mult)
                m2 = cellp.tile([128, 64], F32, tag=f"m2{l}")
                nc.gpsimd.tensor_mul(m2[:], tact[:, 64:128], c_prev[l][:])
                a1 = cellp.tile([128, 64], F32, tag=f"a1{l}")
                nc.vector.tensor_add(a1[:], t1[:], m2[:])
                cn2 = cellp.tile([128, 64], F32, tag=f"cn{l}")
                nc.vector.tensor_add(cn2[:], a1[:], c_prev[l][:])
                tcn = cellp.tile([128, 64], F32, tag=f"tc{l}")
                nc.scalar.activation(tcn[:], cn2[:], AF.Tanh, scale=0.5)
                nc.vector.scalar_tensor_tensor(
                    stg_out, tact[:, 128:192], 1.0, tcn[:],
                    op0=ALU.add, op1=ALU.mult)
                cnew = cellp.tile([128, 64], F32, tag=f"cw{l}")
                nc.gpsimd.tensor_scalar_mul(cnew[:], cn2[:], 0.5)
                c_prev[l] = cnew

            osb_bufs = [None, None]
            osb_views = [None, None]

            def proj_quarter(g, q):
                v0 = 1000 * q
                gh = 4 * g + q
                osv = osb_views[g % 2]
                ps = psP.tile([128, 1024], F32, tag="psP")
                for (o, w) in ((0, 512), (512, 488)):
                    for k in range(16):
                        nc.tensor.matmul(
                            ps[:, o:o + w], osv[:, k, :],
                            woutv[:, k, v0 + o:v0 + o + w],
                            start=(k == 0), stop=False)
                    nc.tensor.matmul(ps[:, o:o + w], ones_s[:, :],
                                     bout_s[:, v0 + o:v0 + o + w],
                                     start=False, stop=True)
                esc = projp.tile([128, 1000], BF16, tag="esc")
                nc.scalar.activation(esc[:], ps[:, 0:1000], AF.Exp,
                                     scale=float(1.0 / SC),
                                     accum_out=s4[:, gh:gh + 1])
                lsb = projp.tile([128, 1000], F16, tag="lsb")
                nc.vector.tensor_scalar_mul(lsb[:], ps[:, 0:1000],
                                            float(1.0 / SC))
                nc.scalar.dma_start(logits_d[g, :, v0:v0 + 1000], lsb[:])

            def ar_batch(b):
                gs0, gs1 = 8 * b, min(8 * b + 8, ngrp)
                nb8 = gs1 - gs0
                s4v = s4[:].rearrange("p (g q) -> p g q", q=4)
                nc.vector.tensor_reduce(
                    s_all[:, gs0:gs1], s4v[:, gs0:gs1, :],
                    axis=mybir.AxisListType.X, op=ALU.add)
                sloc = agp.tile([128, 8], F32, tag="sloc")
                nc.sync.dma_start(sloc[:, 0:nb8], s_all[:, gs0:gs1])
                sglob = agp.tile([128, 8], F32, tag="sglob",
                                 addr_space="Shared")
                nc.gpsimd.collective_compute(
                    "AllReduce", ALU.add, replica_groups=RG,
                    ins=[sloc[:].opt()], outs=[sglob[:].opt()])
                sg_s = arp.tile([128, 8], F32, tag="sgs")
                nc.sync.dma_start(sg_s[:], sglob[:])
                nc.scalar.activation(logz[:, gs0:gs1], sg_s[:, 0:nb8], AF.Ln)

            def pass_b(g):
                for o in (0, 2000):
                    lin = pbp.tile([128, 2000], F16, tag="lin")
                    nc.gpsimd.dma_start(lin[:], logits_d[g, :, o:o + 2000])
                    lout = pbp.tile([128, 2000], F16, tag="lout")
                    nc.vector.tensor_scalar_sub(lout[:], lin[:],
                                                logz[:, g:g + 1])
                    nc.gpsimd.dma_start(
                        out_t[128 * g:128 * (g + 1), o:o + 2000], lout[:])

            g0_chunk(0)
            g0_prefetch(0)
            g0_prefetch(1)

            for t in range(nticks):
                j = t - LAG
                jj = t - 1 - LAG

                if t < ts:
                    ps0 = psA.tile([128, 256], F32, tag="ps0")
                    if t == 0:
                        rhs0 = lambda k: h0i4[:, k, :]
                    else:
                        rhs0 = lambda k: h0r[:, k, (t - 1) % 8, :]
                    hh(whh0v, rhs0, ps0, g0r[:, (t // 8) % 2, t % 8, :])
                    if DBG and t == 0:
                        dc0 = cellp.tile([128, 256], F32, tag="dc0")
                        nc.vector.tensor_copy(dc0[:], ps0[:, 0:256])
                        nc.sync.dma_start(dps0_t[:], dc0[:])
                    cell(0, ps0, stg[t % 2][:].rearrange(
                        "p (l b2) -> p l b2", l=2)[:, 0, :])

                agin = agp.tile([128, 128], BF16, tag="agin")
                nc.sync.dma_start(agin[:], stg[t % 2][:])
                agout = agp.tile([NC, 128, 128], BF16, tag="agout",
                                 addr_space="Shared")
                nc.gpsimd.collective_compute(
                    "AllGather", ALU.bypass, replica_groups=RG,
                    ins=[agin[:].opt()], outs=[agout[:].opt()])

                if t < ts:
                    for jh in (0, 1):
                        nc.sync.dma_start(
                            h0r5[:, :, jh, t % 8, :],
                            agout[:, :, 32 * jh:32 * (jh + 1)].rearrange(
                                "r p b -> p r b"))
                if 0 <= jj < ts:
                    g = jj // 4
                    sl = jj % 4
                    if sl == 0:
                        osb_bufs[g % 2] = osbp.tile([128, 16 * 128], BF16,
                                                    tag="osb", name="osb")
                        osb_views[g % 2] = osb_bufs[g % 2][:].rearrange(
                            "p (k q) -> p k q", k=16)
                    ov5 = osb_bufs[g % 2][:].rearrange(
                        "p (kp jh q) -> p kp jh q", kp=8, jh=2)
                    for jh in (0, 1):
                        nc.scalar.dma_start(
                            ov5[:, :, jh, B * sl:B * (sl + 1)],
                            agout[:, :, 64 + 32 * jh:64 + 32 * (jh + 1)]
                            .rearrange("r p b -> p r b"))

                if DBG and t == LAG:
                    dh0c = cellp.tile([128, 16 * B], F32, tag="dh0c")
                    nc.vector.tensor_copy(
                        dh0c[:].rearrange("p (k b) -> p k b", b=B),
                        h0r[:, :, 0, :])
                    nc.sync.dma_start(dh0_t[:], dh0c[:])

                if 0 <= j < ts and j % D == 0:
                    nb = min(D, ts - j)
                    psg = psG.tile([128, 1024], F32, tag="psg")
                    sl0 = j % 8
                    for m in range(8):
                        for k in range(16):
                            nc.tensor.matmul(
                                psg[:, 128 * m:128 * m + B * nb],
                                wih1v[:, k, 128 * m:128 * (m + 1)],
                                h0r[:, k, sl0:sl0 + nb, :],
                                start=(k == 0), stop=(k == 15))
                    for m in range(8):
                        nc.scalar.activation(
                            g1r[:, 0:nb, 32 * m:32 * (m + 1)],
                            psg[:, 128 * m:128 * m + B * nb],
                            AF.Identity, bias=b1_s[:, m:m + 1])

                if 0 <= j < ts:
                    ps1 = psB.tile([128, 256], F32, tag="ps1")
                    if j == 0:
                        rhs1 = lambda k: h1i4[:, k, :]
                    else:
                        gp = (j - 1) // 4
                        slp = (j - 1) % 4
                        rhs1 = lambda k: osb_views[gp % 2][
                            :, k, B * slp:B * (slp + 1)]
                    hh(whh1v, rhs1, ps1, g1r[:, j % D, :])
                    if DBG and j == 0:
                        dcp = cellp.tile([128, 256], F32, tag="dcp")
                        nc.vector.tensor_copy(dcp[:], ps1[:, 0:256])
                        nc.sync.dma_start(dps1_t[:], dcp[:])
                        dcg = cellp.tile([128, 256], F32, tag="dcg")
                        nc.vector.tensor_copy(dcg[:], g1r[:, 0, :])
                        nc.sync.dma_start(dg1_t[:], dcg[:])
                    cell(1, ps1, stg[(t + 1) % 2][:].rearrange(
                        "p (l b2) -> p l b2", l=2)[:, 1, :])
                    if DBG and j == 0:
                        dch = cellp.tile([128, 64], F32, tag="dch")
                        nc.vector.tensor_copy(
                            dch[:], stg[(t + 1) % 2][:].rearrange(
                                "p (l b2) -> p l b2", l=2)[:, 1, :])
                        nc.sync.dma_start(dh1_t[:], dch[:])

                for (g, q) in proj_inloop.get(t, []):
                    proj_quarter(g, q)

                if t in ar_inloop:
                    ar_batch(ar_inloop[t])
                for g in passb_inloop.get(t, []):
                    pass_b(g)

                if t + 1 < NCH:
                    g0_chunk(t + 1)
                if t % 8 == 7:
                    g0_prefetch(t // 8 + 2)

            for (g, q) in proj_tail:
                proj_quarter(g, q)
            for b in ar_tail:
                ar_batch(b)
                for g in range(8 * b, min(8 * b + 8, ngrp)):
                    pass_b(g)

    nc.compile()
    return nc


_NC_CACHE = {}


def _get_nc(ts):
    if ts not in _NC_CACHE:
        _NC_CACHE[ts] = build_nc(ts)
    return _NC_CACHE[ts]


def run_device(inputs, ts=TS_FULL, **run_kwargs):
    in_maps, ntok_pad = prep_inputs(inputs, ts)
    nc = _get_nc(ts)
    res = bass_utils.run_bass_kernel_spmd(nc, in_maps,
                                          core_ids=list(range(NC)), **run_kwargs)
    ntok = ts * B
    logp = np.empty((ntok, V), np.float32)
    for c in range(NC):
        logp[:, VS * c:VS * (c + 1)] = res.results[c]["out"][:ntok].astype(np.float32)
    out = np.zeros((B, T, V), np.float32)
    out[:, 0, SOS_ID] = 1.0
    out[:, 1:1 + ts, :] = logp.reshape(ts, B, V).transpose(1, 0, 2)
    return out, res


def kernel(**inputs) -> np.ndarray:
    out, _ = run_device(inputs, TS_FULL)
    return out
